# revision 49
# baseline (speedup 1.0000x reference)
"""Trainium2 Bass kernel for nn_Attention_aggregator (B=8, N=4096, F=128, E=128).

Sharding: data-parallel over batch - one batch element per NeuronCore (8 cores).
Each core computes, for its batch b:
    att  = x @ x.T                        [N, N]
    att  = where(adj==0, -9999999, att)
    sm   = softmax(att, axis=-1)
    comb = sm @ x                         [N, F]
    out  = relu(concat([x, comb], -1) @ W.T)      [N, E]

Device decomposition (transposed orientation; contraction of the aggregation
matmul lands on partitions; attention symmetry makes transposed logits free):
    E^T[m, r] = exp(att[m, r] - 80)
    diagonal of att killed in PSUM by an accumulating (-30000*I) @ I matmul
    P^T = E^T * adjT  (adjT int8 in HBM, DMA-cast to int16 in SBUF)
    [S2z | S1] = P^T.T @ [z | 1]  (z = x @ W2^T host-precomputed; ones column
        => row-sum in column E). By linearity comb @ W2^T =
        (ev*S2z + coef*z_r) / (ev*S1 + coef), so no transposes are needed
        anywhere on-chip: out = relu(w1x + (ev*S2z + coef*z)*rden) with
        w1x = x @ W1^T also host-precomputed.

Perf structure (ACT exp of the 16.8M logits is the 133us pace-setter):
  - adjacency stored int8 in HBM (16MB/core), SWDGE DMA casts int8->int16
  - logits accumulate into [128, 3, 512] PSUM groups (3 j-blocks = 3 banks,
    double buffered = 6 banks) so exp runs as one ACTIVATE over FD=1536
  - mask applied as one [128,1536] bf16*int16 tensor_tensor (DVE 2x mode)
  - all inputs host-pre-transposed/pre-cast to bf16 (xt, zb=[z|1], w1x);
    ev/coef/negbig host-computed; zero PE transposes, zero setup ALU work
  - epilogue is pure DVE: relu(w1x + (ev*S2z + coef*z)*rden)
  - Scalar queue carries only the 88 exp ACTIVATEs in steady state
"""

import sys

for _p in ("/opt/trn_rl_repo", "/root/.axon_site/_ro/trn_rl_repo"):
    if _p not in sys.path:
        sys.path.append(_p)

import numpy as np
import ml_dtypes

import concourse.bass as bass
import concourse.mybir as mybir
from concourse import bacc
from concourse.tile import TileContext
from concourse.bass_utils import run_bass_kernel_spmd

F32 = mybir.dt.float32
BF16 = mybir.dt.bfloat16
I16 = mybir.dt.int16
I8 = mybir.dt.int8

B, N, F, E = 8, 4096, 128, 128
RC = 512               # r-chunk width (one PSUM bank of fp32)
NB = N // 128          # 32 m-blocks
NRC = N // RC          # 8 r-chunks
T = RC // 128          # 4 sub-blocks per r-chunk
EXP_BIAS = -80.0

# j-block group sizes per rc sweep (3 PSUM banks per group, double buffered)
GROUPS = [3] * 10 + [2]
assert sum(GROUPS) == NB

_CACHED = {}


def _build():
    nc = bacc.Bacc("TRN2", target_bir_lowering=False, debug=False, num_devices=B)
    xt_d = nc.dram_tensor("xt", [128, NB, 128], BF16, kind="ExternalInput").ap()
    zb_d = nc.dram_tensor("zb", [128, NB, E + 1], BF16, kind="ExternalInput").ap()
    w1x_d = nc.dram_tensor("w1x", [128, NB, E], BF16, kind="ExternalInput").ap()
    adjt_d = nc.dram_tensor("adjt", [N, N], I8, kind="ExternalInput").ap()
    ev_d = nc.dram_tensor("ev", [128, NB], F32, kind="ExternalInput").ap()
    coef_d = nc.dram_tensor("coef", [128, NB], F32, kind="ExternalInput").ap()
    negbig_d = nc.dram_tensor("negbig", [128, 128], BF16, kind="ExternalInput").ap()
    ident_d = nc.dram_tensor("ident", [128, 128], BF16, kind="ExternalInput").ap()
    out_d = nc.dram_tensor("out", [N, E], F32, kind="ExternalOutput").ap()

    adjt_v = adjt_d.rearrange("(o p) c -> p o c", p=128)    # [128, NB, N] int8
    out_v = out_d.rearrange("(o p) e -> p o e", p=128)      # [128, NB, E]

    with TileContext(nc) as tc:
        with (
            tc.tile_pool(name="singles", bufs=1) as singles,
            tc.tile_pool(name="adjrc", bufs=8) as adjrc_pool,
            tc.tile_pool(name="et", bufs=8) as e_pool,
            tc.tile_pool(name="pt", bufs=8) as p_pool,
            tc.tile_pool(name="small", bufs=12) as small,
            tc.tile_pool(name="sc", bufs=2) as sc_pool,
            tc.tile_pool(name="outp", bufs=6) as out_pool,
            tc.tile_pool(name="psumA", bufs=2, space="PSUM") as psum_a,
            tc.tile_pool(name="psumC", bufs=1, space="PSUM") as psum_c,
        ):
            # ---------------- setup: input DMAs only ----------------
            expb = singles.tile([128, 1], F32)
            nc.vector.memset(expb[:], EXP_BIAS)

            # xt on both HWDGE rings, interleaved with zb on the sync ring
            # (quads need zb chunk c well before epilogue inputs); negbig/
            # ident ride first on the scalar ring (group 0 of rc 0 has
            # diagonal blocks, so the very first exp transitively needs them)
            xt_sb = singles.tile([128, NB, 128], BF16)
            zb_sb = singles.tile([128, NB, E + 1], BF16)
            w1x_sb = singles.tile([128, NB, E], BF16)
            ev_sb = singles.tile([128, NB], F32)
            coef_sb = singles.tile([128, NB], F32)
            negbig_bf = singles.tile([128, 128], BF16)
            ident_bf = singles.tile([128, 128], BF16)
            xs_all = singles.tile([128, NB, E], BF16)

            # PE warmup BEFORE the DMA issues (read-before-first-write has no
            # dep): dummy matmuls on zb garbage during the DMA wait push HAM
            # to K=8/8 before the first real logits group. The zb DMA picks
            # up a WAR dep on these, finishing ~16us -- quads need it ~19us.
            warmA = psum_a.tile([128, 3, RC], F32, name="warm", tag="grp")
            for _k in range(6):
                nc.tensor.matmul(warmA[:, 0, :], zb_sb[:, 31, 0:128],
                                 zb_sb[:, 28:32, 0:128], start=True, stop=True,
                                 skip_group_check=True)

            # flat group list: (rc, g, j0, gsz)
            glist = []
            for rc in range(NRC):
                j0 = 0
                for g, gsz in enumerate(GROUPS):
                    glist.append((rc, g, j0, gsz))
                    j0 += gsz
            NG = len(glist)

            # adjacency: one cast-DMA per group (int8 HBM -> int16 SBUF),
            # prefetched PF groups ahead. The FIRST tile's software-cast
            # latency (~8us) gates the whole pipeline: issue the prefetches
            # before any other SWDGE traffic. PF=6 puts tiles 0-5 ahead of
            # the 1MB zb on the serial SWDGE queue -- with PF=3, adj3 landed
            # behind zb (~26us) and stalled mask g3 -> exp for ~6us.
            PF = 6
            adj_tiles = {}

            def issue_adj(i_):
                if i_ >= NG:
                    return
                rc_, _, j0_, gsz_ = glist[i_]
                adjg = adjrc_pool.tile([128, 3, RC], I16, name="adjg")
                nc.gpsimd.dma_start(
                    out=adjg[:, 0:gsz_, :],
                    in_=adjt_v[:, j0_:j0_ + gsz_, rc_ * RC:(rc_ + 1) * RC])
                adj_tiles[i_] = adjg

            for _i in range(PF):
                issue_adj(_i)

            # zb rides SWDGE right behind the 3 adj prefetches (436GB/s
            # there; the in-loop adj issues queue after it and stay ahead of
            # the mask cadence). xt/w1x split across the two ~50GB/s HWDGE
            # rings in consumption order.
            nc.gpsimd.dma_start(out=zb_sb[:], in_=zb_d)
            nc.sync.dma_start(out=xt_sb[:, 0:8, :], in_=xt_d[:, 0:8, :])
            nc.scalar.dma_start(out=negbig_bf[:], in_=negbig_d)
            nc.scalar.dma_start(out=ident_bf[:], in_=ident_d)
            nc.scalar.dma_start(out=xt_sb[:, 8:20, :], in_=xt_d[:, 8:20, :])
            nc.sync.dma_start(out=xt_sb[:, 20:NB, :], in_=xt_d[:, 20:NB, :])
            nc.sync.dma_start(out=ev_sb[:], in_=ev_d)
            nc.sync.dma_start(out=coef_sb[:], in_=coef_d)
            nc.scalar.dma_start(out=w1x_sb[:, 0:16, :], in_=w1x_d[:, 0:16, :])
            nc.sync.dma_start(out=w1x_sb[:, 16:NB, :], in_=w1x_d[:, 16:NB, :])

            # precompute xs = coef*z for every block (no C dependency; the
            # scheduler slots these into early DVE idle time, lightening the
            # per-rc epilogue phases)
            for jj in range(NB):
                nc.vector.tensor_scalar_mul(xs_all[:, jj, :],
                                            zb_sb[:, jj, 0:E],
                                            coef_sb[:, jj:jj + 1])

            # ---------------- main loop ----------------
            LAG = 4
            pending = []   # (rc, g, j0, gsz, pt_tile, c0, c1)

            def emit_quads(item):
                rc_, g_, j0_, gsz_, pt_, c0_, c1_ = item
                for jj in range(gsz_):
                    j = j0_ + jj
                    for t in range(T):
                        if t < 3:
                            outap = c0_[:, t * 129:t * 129 + 129]
                        else:
                            outap = c1_[:, 0:129]
                        nc.tensor.matmul(
                            outap,
                            pt_[:, jj, t * 128:(t + 1) * 128],
                            zb_sb[:, j, 0:E + 1],
                            start=(j == 0 and t in (0, 3)),
                            stop=(j == NB - 1 and t in (2, 3)),
                            skip_group_check=True)

            epi_queue = []   # deferred per-block epilogue phases

            def push_epilogue(rc_, c0_, c1_, last=False):
                # phase 0: copy PSUM quads to SBUF (releases the C banks),
                # batched den + reciprocal; phases 1..4: per-block chain.
                # Phases are drained one per main-loop iteration so the DVE
                # queue interleaves them with the mask tensor_tensors.
                # bf16 staging: keeps every per-block DVE op in a 2x/4x mode
                sc0 = sc_pool.tile([128, 387], BF16, tag="sc0")
                sc1 = sc_pool.tile([128, 129], BF16, tag="sc1")
                dens = small.tile([128, T], F32, tag="dens")
                rdens = small.tile([128, T], F32, tag="rdens")

                def views(t):
                    if t < 3:
                        return (sc0[:, t * 129:t * 129 + 128],
                                sc0[:, t * 129 + 128:t * 129 + 129])
                    return sc1[:, 0:128], sc1[:, 128:129]

                def p0():
                    # for the final rc the exp stream is over: offload the
                    # PSUM copies to the idle Scalar engine so the DVE tail
                    # chain shortens
                    if last:
                        nc.scalar.activation(sc0[:], c0_[:, 0:387],
                                             mybir.ActivationFunctionType.Copy)
                        nc.scalar.activation(sc1[:], c1_[:, 0:129],
                                             mybir.ActivationFunctionType.Copy)
                    else:
                        nc.vector.tensor_copy(sc0[:], c0_[:, 0:387])
                        nc.vector.tensor_copy(sc1[:], c1_[:, 0:129])
                    for t in range(T):
                        blk = rc_ * T + t
                        _, S1 = views(t)
                        nc.vector.scalar_tensor_tensor(
                            dens[:, t:t + 1], S1, ev_sb[:, blk:blk + 1],
                            coef_sb[:, blk:blk + 1],
                            mybir.AluOpType.mult, mybir.AluOpType.add)
                    nc.vector.reciprocal(rdens[:], dens[:])

                def pt_phase(t):
                    blk = rc_ * T + t
                    S2, _ = views(t)
                    evb = ev_sb[:, blk:blk + 1]
                    cfb = coef_sb[:, blk:blk + 1]
                    cu = small.tile([128, E], BF16, tag="cu")
                    nc.vector.scalar_tensor_tensor(
                        cu[:], S2, evb, xs_all[:, blk, :],
                        mybir.AluOpType.mult, mybir.AluOpType.add)
                    otp = small.tile([128, E], BF16, tag="otp")
                    nc.vector.scalar_tensor_tensor(
                        otp[:], cu[:], rdens[:, t:t + 1], w1x_sb[:, blk, :],
                        mybir.AluOpType.mult, mybir.AluOpType.add)
                    ot = out_pool.tile([128, E], F32)
                    if last:
                        nc.scalar.activation(ot[:], otp[:],
                                             mybir.ActivationFunctionType.Relu)
                    else:
                        nc.vector.tensor_relu(ot[:], otp[:])
                    nc.sync.dma_start(out=out_v[:, blk, :], in_=ot[:])

                # p0 runs inline (it releases the C banks for the next rc's
                # quads); the per-block phases spread over later iterations
                p0()
                for t in range(T):
                    epi_queue.append(lambda t=t: pt_phase(t))

            c0_cur = c1_cur = None
            for i, (rc, g, j0, gsz) in enumerate(glist):
                # 1. prefetch adjacency for group i+PF
                issue_adj(i + PF)

                # 3. logits for group i
                if g == 0:
                    c0_cur = psum_c.tile([128, RC], F32, name="C0", tag="C0")
                    c1_cur = psum_c.tile([128, RC], F32, name="C1", tag="C1")
                psA = psum_a.tile([128, 3, RC], F32, name="psA", tag="grp")
                for jj in range(gsz):
                    j = j0 + jj
                    diag = rc * T <= j < (rc + 1) * T
                    nc.tensor.matmul(psA[:, jj, :], xt_sb[:, j, :],
                                     xt_sb[:, rc * T:(rc + 1) * T, :],
                                     start=True, stop=not diag,
                                     skip_group_check=True)
                    if diag:
                        off = (j - rc * T) * 128
                        nc.tensor.matmul(psA[:, jj, off:off + 128],
                                         negbig_bf[:], ident_bf[:],
                                         start=False, stop=True,
                                         skip_group_check=True)

                # 4. exp + mask
                et = e_pool.tile([128, 3, RC], BF16, name="et")
                nc.scalar.activation(et[:, 0:gsz, :], psA[:, 0:gsz, :],
                                     mybir.ActivationFunctionType.Exp,
                                     bias=expb[:])
                pt = p_pool.tile([128, 3, RC], BF16, name="pt")
                adjg = adj_tiles.pop(i)
                nc.vector.tensor_tensor(
                    pt[:, 0:gsz, :], et[:, 0:gsz, :], adjg[:, 0:gsz, :],
                    mybir.AluOpType.mult)

                pending.append((rc, g, j0, gsz, pt, c0_cur, c1_cur))

                # 5. lagged quads AFTER this group's logits: each iteration's
                # logits precede the older quads in the PE FIFO, so a quad
                # stalled on a late input (zb at ~25us) can't block the exp
                # stream's logits behind it. Larger lag in the startup window
                # keeps the zb-dependent quads out of the queue until it has
                # landed; taper near the end shortens the drain tail.
                lag_now = 6 if i < 14 else 5
                lag_now = min(lag_now, max(1, NG - 2 - i))
                while len(pending) > lag_now:
                    item = pending.pop(0)
                    emit_quads(item)
                    if item[1] == len(GROUPS) - 1:
                        push_epilogue(item[0], item[5], item[6],
                                      last=(item[0] == NRC - 1))

                # 6. drain one deferred epilogue phase per iteration
                if epi_queue:
                    epi_queue.pop(0)()

            while pending:
                item = pending.pop(0)
                emit_quads(item)
                if item[1] == len(GROUPS) - 1:
                    push_epilogue(item[0], item[5], item[6],
                                  last=(item[0] == NRC - 1))
            while epi_queue:
                epi_queue.pop(0)()

    nc.compile()
    return nc


def _get_nc():
    if "nc" not in _CACHED:
        _CACHED["nc"] = _build()
    return _CACHED["nc"]


def kernel(**inputs) -> np.ndarray:
    x_all = np.asarray(inputs["node_features"], dtype=np.float32)   # [B, N, F]
    adj_all = np.asarray(inputs["adj_list"])                        # [B, N, N] int32
    W = np.asarray(inputs["W"], dtype=np.float32)                   # [E, 2F]

    W1 = W[:, :F]                                                   # [E, F]
    W2 = W[:, F:]                                                   # [E, F]
    bf16 = ml_dtypes.bfloat16
    negbig = (np.eye(128, dtype=np.float32) * -30000.0).astype(bf16)
    ident = np.eye(128, dtype=np.float32).astype(bf16)

    nc = _get_nc()
    in_maps = []
    for b in range(B):
        x = x_all[b]                                                # [N, F]
        adjt = np.ascontiguousarray(adj_all[b].T.astype(np.int8))
        diag = np.ascontiguousarray(np.diagonal(adj_all[b])).astype(np.float32)
        d = (x * x).sum(-1)                                         # [N]
        ev = np.exp(-diag * np.maximum(0.0, d - 110.0)).astype(np.float32)
        coef = (diag * np.exp(np.minimum(d - 80.0, 30.0))).astype(np.float32)
        z = x @ W2.T                                                # [N, E]
        w1x = x @ W1.T                                              # [N, E]

        xt = np.ascontiguousarray(
            x.reshape(NB, 128, F).transpose(2, 0, 1)).astype(bf16)  # [F, NB, 128]
        zb = np.empty((128, NB, E + 1), dtype=bf16)
        zb[:, :, :E] = z.reshape(NB, 128, E).transpose(1, 0, 2).astype(bf16)
        zb[:, :, E] = bf16(1.0)
        w1xb = np.ascontiguousarray(
            w1x.reshape(NB, 128, E).transpose(1, 0, 2)).astype(bf16)
        in_maps.append({
            "xt": xt,
            "zb": zb,
            "w1x": w1xb,
            "adjt": adjt,
            "ev": np.ascontiguousarray(ev.reshape(NB, 128).T),
            "coef": np.ascontiguousarray(coef.reshape(NB, 128).T),
            "negbig": negbig,
            "ident": ident,
        })

    res = run_bass_kernel_spmd(nc, in_maps, core_ids=list(range(B)))
    out = np.stack([res.results[b]["out"] for b in range(B)], axis=0)
    return out.astype(np.float32, copy=False)


# revision 50
# speedup vs baseline: 1.0023x; 1.0023x over previous
"""Trainium2 Bass kernel for nn_Attention_aggregator (B=8, N=4096, F=128, E=128).

Sharding: data-parallel over batch - one batch element per NeuronCore (8 cores).
Each core computes, for its batch b:
    att  = x @ x.T                        [N, N]
    att  = where(adj==0, -9999999, att)
    sm   = softmax(att, axis=-1)
    comb = sm @ x                         [N, F]
    out  = relu(concat([x, comb], -1) @ W.T)      [N, E]

Device decomposition (transposed orientation; contraction of the aggregation
matmul lands on partitions; attention symmetry makes transposed logits free):
    E^T[m, r] = exp(att[m, r] - 80)
    diagonal of att killed in PSUM by an accumulating (-30000*I) @ I matmul
    P^T = E^T * adjT  (adjT int8 in HBM, DMA-cast to int16 in SBUF)
    [S2z | S1] = P^T.T @ [z | 1]  (z = x @ W2^T host-precomputed; ones column
        => row-sum in column E). By linearity comb @ W2^T =
        (ev*S2z + coef*z_r) / (ev*S1 + coef), so no transposes are needed
        anywhere on-chip: out = relu(w1x + (ev*S2z + coef*z)*rden) with
        w1x = x @ W1^T also host-precomputed.

Perf structure (ACT exp of the 16.8M logits is the 133us pace-setter):
  - adjacency stored int8 in HBM (16MB/core), SWDGE DMA casts int8->int16
  - logits accumulate into [128, 3, 512] PSUM groups (3 j-blocks = 3 banks,
    double buffered = 6 banks) so exp runs as one ACTIVATE over FD=1536
  - mask applied as one [128,1536] bf16*int16 tensor_tensor (DVE 2x mode)
  - all inputs host-pre-transposed/pre-cast to bf16 (xt, zb=[z|1], w1x);
    ev/coef/negbig host-computed; zero PE transposes, zero setup ALU work
  - epilogue is pure DVE: relu(w1x + (ev*S2z + coef*z)*rden)
  - Scalar queue carries only the 88 exp ACTIVATEs in steady state
"""

import sys

for _p in ("/opt/trn_rl_repo", "/root/.axon_site/_ro/trn_rl_repo"):
    if _p not in sys.path:
        sys.path.append(_p)

import numpy as np
import ml_dtypes

import concourse.bass as bass
import concourse.mybir as mybir
from concourse import bacc
from concourse.tile import TileContext
from concourse.bass_utils import run_bass_kernel_spmd

F32 = mybir.dt.float32
BF16 = mybir.dt.bfloat16
I16 = mybir.dt.int16
I8 = mybir.dt.int8

B, N, F, E = 8, 4096, 128, 128
RC = 512               # r-chunk width (one PSUM bank of fp32)
NB = N // 128          # 32 m-blocks
NRC = N // RC          # 8 r-chunks
T = RC // 128          # 4 sub-blocks per r-chunk
EXP_BIAS = -80.0

# j-block group sizes per rc sweep (3 PSUM banks per group, double buffered)
GROUPS = [3] * 10 + [2]
assert sum(GROUPS) == NB

_CACHED = {}


def _build():
    nc = bacc.Bacc("TRN2", target_bir_lowering=False, debug=False, num_devices=B)
    xt_d = nc.dram_tensor("xt", [128, NB, 128], BF16, kind="ExternalInput").ap()
    zb_d = nc.dram_tensor("zb", [128, NB, E + 1], BF16, kind="ExternalInput").ap()
    w1x_d = nc.dram_tensor("w1x", [128, NB, E], BF16, kind="ExternalInput").ap()
    adjt_d = nc.dram_tensor("adjt", [N, N], I8, kind="ExternalInput").ap()
    ev_d = nc.dram_tensor("ev", [128, NB], F32, kind="ExternalInput").ap()
    coef_d = nc.dram_tensor("coef", [128, NB], F32, kind="ExternalInput").ap()
    negbig_d = nc.dram_tensor("negbig", [128, 128], BF16, kind="ExternalInput").ap()
    ident_d = nc.dram_tensor("ident", [128, 128], BF16, kind="ExternalInput").ap()
    out_d = nc.dram_tensor("out", [N, E], F32, kind="ExternalOutput").ap()

    adjt_v = adjt_d.rearrange("(o p) c -> p o c", p=128)    # [128, NB, N] int8
    out_v = out_d.rearrange("(o p) e -> p o e", p=128)      # [128, NB, E]

    with TileContext(nc) as tc:
        with (
            tc.tile_pool(name="singles", bufs=1) as singles,
            tc.tile_pool(name="adjrc", bufs=8) as adjrc_pool,
            tc.tile_pool(name="et", bufs=8) as e_pool,
            tc.tile_pool(name="pt", bufs=8) as p_pool,
            tc.tile_pool(name="small", bufs=12) as small,
            tc.tile_pool(name="sc", bufs=2) as sc_pool,
            tc.tile_pool(name="outp", bufs=6) as out_pool,
            tc.tile_pool(name="psumA", bufs=2, space="PSUM") as psum_a,
            tc.tile_pool(name="psumC", bufs=1, space="PSUM") as psum_c,
        ):
            # ---------------- setup: input DMAs only ----------------
            expb = singles.tile([128, 1], F32)
            nc.vector.memset(expb[:], EXP_BIAS)

            # xt on both HWDGE rings, interleaved with zb on the sync ring
            # (quads need zb chunk c well before epilogue inputs); negbig/
            # ident ride first on the scalar ring (group 0 of rc 0 has
            # diagonal blocks, so the very first exp transitively needs them)
            xt_sb = singles.tile([128, NB, 128], BF16)
            zb_sb = singles.tile([128, NB, E + 1], BF16)
            w1x_sb = singles.tile([128, NB, E], BF16)
            ev_sb = singles.tile([128, NB], F32)
            coef_sb = singles.tile([128, NB], F32)
            negbig_bf = singles.tile([128, 128], BF16)
            ident_bf = singles.tile([128, 128], BF16)
            xs_all = singles.tile([128, NB, E], BF16)

            # PE warmup BEFORE the DMA issues (read-before-first-write has no
            # dep): dummy matmuls on zb garbage during the DMA wait push HAM
            # to K=8/8 before the first real logits group. The zb DMA picks
            # up a WAR dep on these, finishing ~16us -- quads need it ~19us.
            warmA = psum_a.tile([128, 3, RC], F32, name="warm", tag="grp")
            for _k in range(6):
                nc.tensor.matmul(warmA[:, 0, :], zb_sb[:, 31, 0:128],
                                 zb_sb[:, 28:32, 0:128], start=True, stop=True,
                                 skip_group_check=True)

            # flat group list: (rc, g, j0, gsz)
            glist = []
            for rc in range(NRC):
                j0 = 0
                for g, gsz in enumerate(GROUPS):
                    glist.append((rc, g, j0, gsz))
                    j0 += gsz
            NG = len(glist)

            # adjacency: one cast-DMA per group (int8 HBM -> int16 SBUF),
            # prefetched PF groups ahead. The FIRST tile's software-cast
            # latency (~8us) gates the whole pipeline: issue the prefetches
            # before any other SWDGE traffic. PF=6 puts tiles 0-5 ahead of
            # the 1MB zb on the serial SWDGE queue -- with PF=3, adj3 landed
            # behind zb (~26us) and stalled mask g3 -> exp for ~6us.
            PF = 6
            adj_tiles = {}

            def issue_adj(i_):
                if i_ >= NG:
                    return
                rc_, _, j0_, gsz_ = glist[i_]
                adjg = adjrc_pool.tile([128, 3, RC], I16, name="adjg")
                nc.gpsimd.dma_start(
                    out=adjg[:, 0:gsz_, :],
                    in_=adjt_v[:, j0_:j0_ + gsz_, rc_ * RC:(rc_ + 1) * RC])
                adj_tiles[i_] = adjg

            for _i in range(PF):
                issue_adj(_i)

            # zb rides SWDGE right behind the 3 adj prefetches (436GB/s
            # there; the in-loop adj issues queue after it and stay ahead of
            # the mask cadence). xt/w1x split across the two ~50GB/s HWDGE
            # rings in consumption order.
            # zb head on sync (lands ~19.6us, before quads g0-g2 need it at
            # ~20.6; the SWDGE tail behind the casts was landing ~27 and the
            # exp fences' cumulative PE-counter semantics stalled the whole
            # stream on it); zb tail stays behind the casts on SWDGE
            nc.gpsimd.dma_start(out=zb_sb[:, 10:NB, :], in_=zb_d[:, 10:NB, :])
            nc.sync.dma_start(out=xt_sb[:, 0:8, :], in_=xt_d[:, 0:8, :])
            nc.sync.dma_start(out=zb_sb[:, 0:10, :], in_=zb_d[:, 0:10, :])
            nc.scalar.dma_start(out=negbig_bf[:], in_=negbig_d)
            nc.scalar.dma_start(out=ident_bf[:], in_=ident_d)
            nc.scalar.dma_start(out=xt_sb[:, 8:20, :], in_=xt_d[:, 8:20, :])
            nc.scalar.dma_start(out=xt_sb[:, 20:NB, :], in_=xt_d[:, 20:NB, :])
            nc.sync.dma_start(out=ev_sb[:], in_=ev_d)
            nc.sync.dma_start(out=coef_sb[:], in_=coef_d)
            nc.sync.dma_start(out=w1x_sb[:, 0:16, :], in_=w1x_d[:, 0:16, :])
            nc.sync.dma_start(out=w1x_sb[:, 16:NB, :], in_=w1x_d[:, 16:NB, :])

            # precompute xs = coef*z for every block (no C dependency; the
            # scheduler slots these into early DVE idle time, lightening the
            # per-rc epilogue phases)
            for jj in range(NB):
                nc.vector.tensor_scalar_mul(xs_all[:, jj, :],
                                            zb_sb[:, jj, 0:E],
                                            coef_sb[:, jj:jj + 1])

            # ---------------- main loop ----------------
            LAG = 4
            pending = []   # (rc, g, j0, gsz, pt_tile, c0, c1)

            def emit_quads(item):
                rc_, g_, j0_, gsz_, pt_, c0_, c1_ = item
                for jj in range(gsz_):
                    j = j0_ + jj
                    for t in range(T):
                        if t < 3:
                            outap = c0_[:, t * 129:t * 129 + 129]
                        else:
                            outap = c1_[:, 0:129]
                        nc.tensor.matmul(
                            outap,
                            pt_[:, jj, t * 128:(t + 1) * 128],
                            zb_sb[:, j, 0:E + 1],
                            start=(j == 0 and t in (0, 3)),
                            stop=(j == NB - 1 and t in (2, 3)),
                            skip_group_check=True)

            epi_queue = []   # deferred per-block epilogue phases

            def push_epilogue(rc_, c0_, c1_, last=False):
                # phase 0: copy PSUM quads to SBUF (releases the C banks),
                # batched den + reciprocal; phases 1..4: per-block chain.
                # Phases are drained one per main-loop iteration so the DVE
                # queue interleaves them with the mask tensor_tensors.
                # bf16 staging: keeps every per-block DVE op in a 2x/4x mode
                sc0 = sc_pool.tile([128, 387], BF16, tag="sc0")
                sc1 = sc_pool.tile([128, 129], BF16, tag="sc1")
                dens = small.tile([128, T], F32, tag="dens")
                rdens = small.tile([128, T], F32, tag="rdens")

                def views(t):
                    if t < 3:
                        return (sc0[:, t * 129:t * 129 + 128],
                                sc0[:, t * 129 + 128:t * 129 + 129])
                    return sc1[:, 0:128], sc1[:, 128:129]

                def p0():
                    # for the final rc the exp stream is over: offload the
                    # PSUM copies to the idle Scalar engine so the DVE tail
                    # chain shortens
                    if last:
                        nc.scalar.activation(sc0[:], c0_[:, 0:387],
                                             mybir.ActivationFunctionType.Copy)
                        nc.scalar.activation(sc1[:], c1_[:, 0:129],
                                             mybir.ActivationFunctionType.Copy)
                    else:
                        nc.vector.tensor_copy(sc0[:], c0_[:, 0:387])
                        nc.vector.tensor_copy(sc1[:], c1_[:, 0:129])
                    for t in range(T):
                        blk = rc_ * T + t
                        _, S1 = views(t)
                        nc.vector.scalar_tensor_tensor(
                            dens[:, t:t + 1], S1, ev_sb[:, blk:blk + 1],
                            coef_sb[:, blk:blk + 1],
                            mybir.AluOpType.mult, mybir.AluOpType.add)
                    nc.vector.reciprocal(rdens[:], dens[:])

                def pt_phase(t):
                    blk = rc_ * T + t
                    S2, _ = views(t)
                    evb = ev_sb[:, blk:blk + 1]
                    cfb = coef_sb[:, blk:blk + 1]
                    cu = small.tile([128, E], BF16, tag="cu")
                    nc.vector.scalar_tensor_tensor(
                        cu[:], S2, evb, xs_all[:, blk, :],
                        mybir.AluOpType.mult, mybir.AluOpType.add)
                    otp = small.tile([128, E], BF16, tag="otp")
                    nc.vector.scalar_tensor_tensor(
                        otp[:], cu[:], rdens[:, t:t + 1], w1x_sb[:, blk, :],
                        mybir.AluOpType.mult, mybir.AluOpType.add)
                    ot = out_pool.tile([128, E], F32)
                    if last:
                        nc.scalar.activation(ot[:], otp[:],
                                             mybir.ActivationFunctionType.Relu)
                    else:
                        nc.vector.tensor_relu(ot[:], otp[:])
                    nc.sync.dma_start(out=out_v[:, blk, :], in_=ot[:])

                # p0 runs inline (it releases the C banks for the next rc's
                # quads); the per-block phases spread over later iterations
                p0()
                for t in range(T):
                    epi_queue.append(lambda t=t: pt_phase(t))

            c0_cur = c1_cur = None
            for i, (rc, g, j0, gsz) in enumerate(glist):
                # 1. prefetch adjacency for group i+PF
                issue_adj(i + PF)

                # 3. logits for group i
                if g == 0:
                    c0_cur = psum_c.tile([128, RC], F32, name="C0", tag="C0")
                    c1_cur = psum_c.tile([128, RC], F32, name="C1", tag="C1")
                psA = psum_a.tile([128, 3, RC], F32, name="psA", tag="grp")
                for jj in range(gsz):
                    j = j0 + jj
                    diag = rc * T <= j < (rc + 1) * T
                    nc.tensor.matmul(psA[:, jj, :], xt_sb[:, j, :],
                                     xt_sb[:, rc * T:(rc + 1) * T, :],
                                     start=True, stop=not diag,
                                     skip_group_check=True)
                    if diag:
                        off = (j - rc * T) * 128
                        nc.tensor.matmul(psA[:, jj, off:off + 128],
                                         negbig_bf[:], ident_bf[:],
                                         start=False, stop=True,
                                         skip_group_check=True)

                # 4. exp + mask
                et = e_pool.tile([128, 3, RC], BF16, name="et")
                nc.scalar.activation(et[:, 0:gsz, :], psA[:, 0:gsz, :],
                                     mybir.ActivationFunctionType.Exp,
                                     bias=expb[:])
                pt = p_pool.tile([128, 3, RC], BF16, name="pt")
                adjg = adj_tiles.pop(i)
                nc.vector.tensor_tensor(
                    pt[:, 0:gsz, :], et[:, 0:gsz, :], adjg[:, 0:gsz, :],
                    mybir.AluOpType.mult)

                pending.append((rc, g, j0, gsz, pt, c0_cur, c1_cur))

                # 5. lagged quads AFTER this group's logits: each iteration's
                # logits precede the older quads in the PE FIFO, so a quad
                # stalled on a late input (zb at ~25us) can't block the exp
                # stream's logits behind it. Larger lag in the startup window
                # keeps the zb-dependent quads out of the queue until it has
                # landed; taper near the end shortens the drain tail.
                lag_now = 6 if i < 14 else 5
                lag_now = min(lag_now, max(1, NG - 2 - i))
                while len(pending) > lag_now:
                    item = pending.pop(0)
                    emit_quads(item)
                    if item[1] == len(GROUPS) - 1:
                        push_epilogue(item[0], item[5], item[6],
                                      last=(item[0] == NRC - 1))

                # 6. drain one deferred epilogue phase per iteration
                if epi_queue:
                    epi_queue.pop(0)()

            while pending:
                item = pending.pop(0)
                emit_quads(item)
                if item[1] == len(GROUPS) - 1:
                    push_epilogue(item[0], item[5], item[6],
                                  last=(item[0] == NRC - 1))
            while epi_queue:
                epi_queue.pop(0)()

    nc.compile()
    return nc


def _get_nc():
    if "nc" not in _CACHED:
        _CACHED["nc"] = _build()
    return _CACHED["nc"]


def kernel(**inputs) -> np.ndarray:
    x_all = np.asarray(inputs["node_features"], dtype=np.float32)   # [B, N, F]
    adj_all = np.asarray(inputs["adj_list"])                        # [B, N, N] int32
    W = np.asarray(inputs["W"], dtype=np.float32)                   # [E, 2F]

    W1 = W[:, :F]                                                   # [E, F]
    W2 = W[:, F:]                                                   # [E, F]
    bf16 = ml_dtypes.bfloat16
    negbig = (np.eye(128, dtype=np.float32) * -30000.0).astype(bf16)
    ident = np.eye(128, dtype=np.float32).astype(bf16)

    nc = _get_nc()
    in_maps = []
    for b in range(B):
        x = x_all[b]                                                # [N, F]
        adjt = np.ascontiguousarray(adj_all[b].T.astype(np.int8))
        diag = np.ascontiguousarray(np.diagonal(adj_all[b])).astype(np.float32)
        d = (x * x).sum(-1)                                         # [N]
        ev = np.exp(-diag * np.maximum(0.0, d - 110.0)).astype(np.float32)
        coef = (diag * np.exp(np.minimum(d - 80.0, 30.0))).astype(np.float32)
        z = x @ W2.T                                                # [N, E]
        w1x = x @ W1.T                                              # [N, E]

        xt = np.ascontiguousarray(
            x.reshape(NB, 128, F).transpose(2, 0, 1)).astype(bf16)  # [F, NB, 128]
        zb = np.empty((128, NB, E + 1), dtype=bf16)
        zb[:, :, :E] = z.reshape(NB, 128, E).transpose(1, 0, 2).astype(bf16)
        zb[:, :, E] = bf16(1.0)
        w1xb = np.ascontiguousarray(
            w1x.reshape(NB, 128, E).transpose(1, 0, 2)).astype(bf16)
        in_maps.append({
            "xt": xt,
            "zb": zb,
            "w1x": w1xb,
            "adjt": adjt,
            "ev": np.ascontiguousarray(ev.reshape(NB, 128).T),
            "coef": np.ascontiguousarray(coef.reshape(NB, 128).T),
            "negbig": negbig,
            "ident": ident,
        })

    res = run_bass_kernel_spmd(nc, in_maps, core_ids=list(range(B)))
    out = np.stack([res.results[b]["out"] for b in range(B)], axis=0)
    return out.astype(np.float32, copy=False)
